# revision 7
# baseline (speedup 1.0000x reference)
"""Bahdanau additive attention for Trainium2, 8-core data-parallel.

Reference computation (per batch b):
    dec_t = dec @ W_w.T + W_b                        # (T_dec, H)
    score[d,e] = V_w . tanh(dec_t[d,:] + enc[e,:])   # (T_dec, T_enc)
    attn = softmax(score, axis=e)                    # V_b shift is a no-op
    context = attn @ enc                             # (T_dec, H)

Strategy per core (256 decoder rows each):
  - Keep enc^T [h=128, T_enc] resident in SBUF.
  - Per decoder row d: one ScalarE ACTIVATE computes
    tanh(enc^T + bias=dec_t[:,d]) -> [128, 1024] bf16 tile
    (the add is fused into ACT's per-partition bias).
  - V-dot via TensorE: stationary operand = a sliding window over a
    [128, 256] buffer holding V at column 128, so lhsT_r has V in column r
    and zeros elsewhere; matmul accumulates row r of the [128 d, 512 e]
    PSUM score tile (128-deep accumulation group per PSUM bank).
  - exp on ScalarE straight out of PSUM (no max subtraction needed:
    |score| <= ||V||_1 ~ 9, well inside fp32 exp range; softmax is
    shift-invariant so this matches the reference mathematically).
  - transpose exp via TensorE, context matmul with enc augmented by a
    ones column so the same accumulation yields sum(exp) in column 128;
    normalize with VectorE reciprocal + per-partition scale.

Walrus's LDWEIGHTS lowering holds only one sync-wait slot, so every
matmul operand is staged so that all of a matmul's dependencies land on
a single processor (usually ACT): constants arrive in ONE packed DMA and
are re-staged through ScalarE copies before any TensorE use.
"""

import numpy as np

B, T_DEC, T_ENC, H = 4, 512, 1024, 128
N_CORES = 8
ROWS = B * T_DEC // N_CORES  # 256 decoder rows per core
NBLK = ROWS // 128           # 2 blocks of 128 rows
NCH = T_ENC // H             # 8 encoder chunks of 128

# packed constant layout (columns in the single fp32 input)
O_DECT = 0                    # [H, ROWS]    dec shard, transposed
O_WT = O_DECT + ROWS          # [H, H]       W_w^T
O_WB = O_WT + H               # [H, 1]       W_b
O_ENCT = O_WB + 1             # [H, T_ENC]   enc^T
O_EAUG = O_ENCT + T_ENC       # [H, NCH*(H+1)] enc chunks + ones col
O_IDENT = O_EAUG + NCH * (H + 1)  # [H, H]   identity
O_VWIN = O_IDENT + H          # [H, 2H]      V window (V at col H)
CST_COLS = O_VWIN + 2 * H

_CACHE = {}


def _build_program(nreps=0):
    """nreps=0: straight-line program (the real kernel). nreps>0: wrap the
    main body in a hardware For_i loop repeating it nreps times (for
    wall-clock slope timing)."""
    from contextlib import ExitStack

    import concourse.bacc as bacc
    import concourse.tile as tile
    import concourse.mybir as mybir

    dt = mybir.dt
    AF = mybir.ActivationFunctionType

    nc = bacc.Bacc("TRN2", target_bir_lowering=False, debug=False)

    cst_d = nc.dram_tensor("cst", [H, CST_COLS], dt.float32, kind="ExternalInput").ap()
    out_d = nc.dram_tensor("out", [ROWS, H], dt.float32, kind="ExternalOutput").ap()

    with ExitStack() as ctx:
        tc = ctx.enter_context(tile.TileContext(nc))
        consts = ctx.enter_context(tc.tile_pool(name="consts", bufs=1))
        tanh_pool = ctx.enter_context(tc.tile_pool(name="tanh", bufs=4))
        exp_pool = ctx.enter_context(tc.tile_pool(name="exp", bufs=2))
        expT_pool = ctx.enter_context(tc.tile_pool(name="expT", bufs=2))
        out_pool = ctx.enter_context(tc.tile_pool(name="outp", bufs=2))
        small_pool = ctx.enter_context(tc.tile_pool(name="small", bufs=2))
        score_pool = ctx.enter_context(tc.tile_pool(name="score", bufs=2, space="PSUM"))
        tp_pool = ctx.enter_context(tc.tile_pool(name="tp", bufs=2, space="PSUM"))
        ctx_pool = ctx.enter_context(tc.tile_pool(name="ctxp", bufs=1, space="PSUM"))

        # ---- one packed constant DMA, then ACT/DVE staging copies ----
        cst_sb = consts.tile([H, CST_COLS], dt.float32)
        nc.sync.dma_start(cst_sb[:], cst_d)

        decT_sb = consts.tile([H, ROWS], dt.float32)
        nc.scalar.copy(decT_sb[:], cst_sb[:, O_DECT : O_DECT + ROWS])
        wT_sb = consts.tile([H, H], dt.float32)
        nc.scalar.copy(wT_sb[:], cst_sb[:, O_WT : O_WT + H])
        ident_sb = consts.tile([H, H], dt.float32)
        nc.scalar.copy(ident_sb[:], cst_sb[:, O_IDENT : O_IDENT + H])
        eaug_sb = consts.tile([H, NCH * (H + 1)], dt.float32)
        nc.scalar.copy(eaug_sb[:], cst_sb[:, O_EAUG : O_EAUG + NCH * (H + 1)])
        vwin_sb = consts.tile([H, 2 * H], dt.bfloat16)
        nc.scalar.copy(vwin_sb[:], cst_sb[:, O_VWIN : O_VWIN + 2 * H])
        encT_sb = consts.tile([H, T_ENC], dt.float32)
        nc.vector.tensor_copy(encT_sb[:], cst_sb[:, O_ENCT : O_ENCT + T_ENC])

        # ---- dec_t = W_w @ dec^T + W_b  (layout [h_out=128, d]) ----
        dect_ps = ctx_pool.tile([H, ROWS], dt.float32, tag="dect")
        nc.tensor.matmul(
            dect_ps[:], lhsT=wT_sb[:], rhs=decT_sb[:], start=True, stop=True
        )
        dect_sb = consts.tile([H, ROWS], dt.float32)
        nc.vector.tensor_scalar_add(
            dect_sb[:], dect_ps[:], cst_sb[:, O_WB : O_WB + 1]
        )

        # ---- main loop over 128-row blocks ----
        loop_cm = tc.For_i(0, nreps, 1) if nreps else None
        if loop_cm is not None:
            loop_cm.__enter__()
        for blk in range(NBLK):
            score_ps = score_pool.tile([128, T_ENC], dt.float32)
            for r in range(128):
                d = blk * 128 + r
                th = tanh_pool.tile([H, T_ENC], dt.bfloat16)
                nc.scalar.activation(
                    th[:], encT_sb[:], AF.Tanh, bias=dect_sb[:, d : d + 1], scale=1.0
                )
                for h2 in range(2):
                    nc.tensor.matmul(
                        score_ps[:, h2 * 512 : (h2 + 1) * 512],
                        lhsT=vwin_sb[:, H - r : 2 * H - r],
                        rhs=th[:, h2 * 512 : (h2 + 1) * 512],
                        start=(r == 0),
                        stop=(r == 127),
                    )

            exp_sb = exp_pool.tile([128, T_ENC], dt.float32)
            nc.scalar.activation(exp_sb[:], score_ps[:], AF.Exp)

            expT_sb = expT_pool.tile([128, T_ENC], dt.float32)
            for c in range(NCH):
                tp = tp_pool.tile([128, H], dt.float32)
                nc.tensor.transpose(tp[:], exp_sb[:, c * H : (c + 1) * H], ident_sb[:])
                nc.scalar.copy(expT_sb[:, c * H : (c + 1) * H], tp[:])

            ctx_ps = ctx_pool.tile([128, H + 1], dt.float32, tag="ctx")
            for c in range(NCH):
                nc.tensor.matmul(
                    ctx_ps[:],
                    lhsT=expT_sb[:, c * H : (c + 1) * H],
                    rhs=eaug_sb[:, c * (H + 1) : (c + 1) * (H + 1)],
                    start=(c == 0),
                    stop=(c == NCH - 1),
                )
            ctxo_sb = out_pool.tile([128, H + 1], dt.float32, tag="ctxo")
            nc.scalar.copy(ctxo_sb[:], ctx_ps[:])

            recip = small_pool.tile([128, 1], dt.float32)
            nc.vector.reciprocal(recip[:], ctxo_sb[:, H : H + 1])
            out_sb = out_pool.tile([128, H], dt.float32, tag="out")
            nc.vector.tensor_scalar_mul(out_sb[:], ctxo_sb[:, 0:H], recip[:])
            nc.sync.dma_start(out_d[blk * 128 : (blk + 1) * 128, :], out_sb[:])

        if loop_cm is not None:
            loop_cm.__exit__(None, None, None)

    nc.compile()
    return nc


def _get_program(nreps=0):
    if nreps not in _CACHE:
        _CACHE[nreps] = _build_program(nreps)
    return _CACHE[nreps]


def _host_inputs(inputs):
    dec = np.asarray(inputs["decoder_outputs"], dtype=np.float32)
    enc = np.asarray(inputs["encoder_outputs"], dtype=np.float32)
    W_w = np.asarray(inputs["W_w"], dtype=np.float32)
    W_b = np.asarray(inputs["W_b"], dtype=np.float32)
    V_w = np.asarray(inputs["V_w"], dtype=np.float32)
    # V_b shifts every score equally; softmax is invariant to it.

    dec_flat = dec.reshape(B * T_DEC, H)
    ones = np.ones((T_ENC, 1), dtype=np.float32)

    in_maps = []
    for c in range(N_CORES):
        b = (c * ROWS) // T_DEC
        encb = enc[b]
        cst = np.zeros((H, CST_COLS), dtype=np.float32)
        cst[:, O_DECT : O_DECT + ROWS] = dec_flat[c * ROWS : (c + 1) * ROWS].T
        cst[:, O_WT : O_WT + H] = W_w.T
        cst[:, O_WB] = W_b
        cst[:, O_ENCT : O_ENCT + T_ENC] = encb.T
        eaug = np.concatenate([encb, ones], axis=1)  # (T_ENC, H+1)
        cst[:, O_EAUG : O_EAUG + NCH * (H + 1)] = (
            eaug.reshape(NCH, H, H + 1).transpose(1, 0, 2).reshape(H, NCH * (H + 1))
        )
        cst[:, O_IDENT : O_IDENT + H] = np.eye(H, dtype=np.float32)
        cst[:, O_VWIN + H] = V_w
        in_maps.append({"cst": cst})
    return in_maps


def kernel(**inputs) -> np.ndarray:
    from concourse.bass_utils import run_bass_kernel_spmd

    nc = _get_program()
    in_maps = _host_inputs(inputs)
    res = run_bass_kernel_spmd(nc, in_maps, core_ids=list(range(N_CORES)))
    out = np.concatenate(
        [np.asarray(res.results[c]["out"]) for c in range(N_CORES)], axis=0
    )
    return out.reshape(B, T_DEC, H).astype(np.float32)
